# revision 3
# baseline (speedup 1.0000x reference)
"""PLIF (parametric LIF) spiking layer on 8 Trainium2 NeuronCores.

Computation: y = x @ W.T + b over [T=64, B=256, Cin=1024] -> Cout=1024, then a
per-timestep PLIF recurrence v = v + (y_t - v)*sigmoid(w); spike = (v >= 1);
hard reset v *= (1-spike). Output = spikes [T, B, Cout] fp32.

Strategy (see repo notes):
- Data-parallel over batch: core c handles b in [32c, 32c+32).
- Scaled recurrence: u_t = v_t * a^-t (a = 1-sigmoid(w)). Then u_t = u'_{t-1} + z_t
  with z_t = d*a^-t*y_t, spike iff u_t >= a^-t, reset u'_t = 0. The d*a^-t factor
  is folded into x columns on the host (exact powers of 2 when w=0), so the
  device step is ONE fused custom-DVE op:
      u' = select(u + z >= th_t, 0, u + z)
- Spikes are derived on the host as (u' == 0) - exact except measure-zero
  coincidences, which also leave the state unchanged.
- Matmul layout: out[chan, n=t*32+b] = W^T.T @ x^T. W^T resident in SBUF;
  x^T (host-pretransposed+scaled) streamed in 512-column groups; fp32r PE mode
  (1 cycle/row); PSUM evicted to a z-buffer by the scalar engine.
"""

import numpy as np

T, B, CIN, COUT = 64, 256, 1024, 1024
NCORES = 8
BSH = B // NCORES          # 32 batch rows per core
NROWS = T * BSH            # 2048 matmul rows per core
NGROUPS = 4                # n-tile groups of 512 rows (16 timesteps each)
NG = NROWS // NGROUPS      # 512
TPG = NG // BSH            # 16 timesteps per group
KC = CIN // 128            # 8 contraction chunks
GC = COUT // 128           # 8 output-channel chunks
SFREE = GC * BSH           # 256 = state free size

_CACHE = {}


def _make_lif_op():
    import concourse.dve_ops as dve_ops
    from concourse.dve_ops import DveOp, OPS
    from concourse.dve_spec import Spec, Src0, Src1, Zero, C0, lower, select, _has_src1
    from concourse.dve_uop import DveOpSpec

    name = "LIF_STEP_ANT"
    for op in OPS:
        if op.name == name:
            return op
    w_ = Src0 + Src1
    spec = Spec(
        body=select(w_ >= C0, Zero, w_),
        reference=lambda in0, in1, s0, s1, imm2: np.where(
            (in0 + in1) >= s0, 0.0, in0 + in1
        ).astype(np.float32),
    )
    row = dve_ops._CUSTOM_DVE_ROW_BASE + len(OPS)
    assert row < 0x20
    shas = {}
    for ver in ("v3", "v4"):
        tmp = DveOpSpec(name=name, opcode=row, uops=lower(spec, ver=ver),
                        rd1_en=_has_src1(spec))
        shas[ver] = tmp.sha(ver)
    op = DveOp(name, spec, subdim=False, uops_sha=shas)
    OPS.append(op)
    dve_ops._SUB_OPCODE_FOR_NAME[name] = row
    dve_ops.CUSTOM_DVE_SPECS[name] = spec
    return op


def _build(thresholds, mm_dtype_name="float32r", u_bufs=6, x_bufs=2, z_bufs=2,
           psum_bufs=4):
    import concourse.bacc as bacc
    import concourse.mybir as mybir
    import concourse.tile as tile
    from contextlib import ExitStack

    LIF = _make_lif_op()
    mm_dt = getattr(mybir.dt, mm_dtype_name)
    f32 = mybir.dt.float32

    nc = bacc.Bacc("TRN2", target_bir_lowering=False, debug=False)
    xT_d = nc.declare_dram_parameter("xT", [CIN, NROWS], f32, isOutput=False)
    WT_d = nc.declare_dram_parameter("WT", [CIN, COUT], f32, isOutput=False)
    u_d = nc.declare_dram_parameter("u_out", [T, 128, SFREE], f32, isOutput=True)

    with tile.TileContext(nc) as tc:
        with ExitStack() as ctx:
            wp = ctx.enter_context(tc.tile_pool(name="wp", bufs=1))
            xp = ctx.enter_context(tc.tile_pool(name="xp", bufs=x_bufs))
            zp = ctx.enter_context(tc.tile_pool(name="zp", bufs=z_bufs))
            up = ctx.enter_context(tc.tile_pool(name="up", bufs=u_bufs))
            ip = ctx.enter_context(tc.tile_pool(name="ip", bufs=1))
            pp = ctx.enter_context(tc.tile_pool(name="pp", bufs=psum_bufs, space="PSUM"))

            # Resident W^T: [128 part(k), KC kchunk, COUT]
            wt = wp.tile([128, KC, COUT], mm_dt, tag="wt")
            nc.sync.dma_start(
                wt[:], WT_d.ap().rearrange("(c p) o -> p c o", p=128).bitcast(mm_dt))

            u_prev = ip.tile([128, GC, BSH], f32, tag="u0")
            nc.vector.memset(u_prev[:], 0.0)

            for ng in range(NGROUPS):
                xt = xp.tile([128, KC, NG], mm_dt, tag="xt")
                nc.sync.dma_start(
                    xt[:],
                    xT_d.ap()[:, ng * NG:(ng + 1) * NG]
                    .rearrange("(c p) n -> p c n", p=128).bitcast(mm_dt))

                zbuf = zp.tile([128, GC * NG], f32, tag="zbuf")
                for g in range(GC):
                    psum = pp.tile([128, NG], f32, tag="ps")
                    for kc in range(KC):
                        nc.tensor.matmul(
                            psum[:],
                            wt[:, kc, g * 128:(g + 1) * 128],
                            xt[:, kc, :],
                            start=(kc == 0), stop=(kc == KC - 1))
                    nc.scalar.copy(zbuf[:, g * NG:(g + 1) * NG], psum[:])

                zv = zbuf[:].rearrange("p (g n) -> p g n", g=GC)
                for ti in range(TPG):
                    t = ng * TPG + ti
                    u_new = up.tile([128, GC, BSH], f32, tag="u")
                    z_ap = zv[:, :, ti * BSH:(ti + 1) * BSH]
                    nc.vector._custom_dve(
                        LIF, out=u_new[:], in0=u_prev[:], in1=z_ap,
                        s0=float(thresholds[t]))
                    nc.sync.dma_start(
                        u_d.ap()[t].rearrange("p (g n) -> p g n", g=GC), u_new[:])
                    u_prev = u_new
    nc.compile()
    return nc


def _get_nc(key, thresholds, mm_dtype_name):
    if key not in _CACHE:
        _CACHE[key] = _build(thresholds, mm_dtype_name=mm_dtype_name)
    return _CACHE[key]


def kernel(x, W, b, w, _trace=False, _mm_dtype="float32r"):
    from concourse.bass_utils import run_bass_kernel_spmd

    x = np.ascontiguousarray(np.asarray(x, dtype=np.float32))
    W = np.ascontiguousarray(np.asarray(W, dtype=np.float32))
    b = np.asarray(b, dtype=np.float32)
    wv = float(np.asarray(w, dtype=np.float32))
    assert x.shape == (T, B, CIN) and W.shape == (COUT, CIN)
    assert not np.any(b), "nonzero bias not implemented (spec fills zeros)"

    d = np.float64(1.0) / (np.float64(1.0) + np.exp(np.float64(-wv)))
    a = np.float64(1.0) - d
    # z scale per timestep: d * a^-t ; threshold: a^-t  (exact powers of 2 at w=0)
    tt = np.arange(T, dtype=np.float64)
    scales = (d * a ** (-tt)).astype(np.float32)
    thresholds = (a ** (-tt)).astype(np.float32)
    assert np.all(np.isfinite(scales)) and np.all(np.isfinite(thresholds))

    key = (_mm_dtype, wv)
    nc = _get_nc(key, thresholds, _mm_dtype)

    # Host prep: per-core x'^T = (x[:, 32c:32c+32, :] * scale_t).reshape(2048,1024).T
    xs = x * scales[:, None, None]            # [T, B, CIN] fp32 (exact *2^k at w=0)
    WT = np.ascontiguousarray(W.T)            # [CIN, COUT]
    in_maps = []
    for c in range(NCORES):
        xc = xs[:, c * BSH:(c + 1) * BSH, :].reshape(NROWS, CIN)
        in_maps.append({"xT": np.ascontiguousarray(xc.T), "WT": WT})

    res = run_bass_kernel_spmd(nc, in_maps, list(range(NCORES)), trace=_trace)

    out = np.empty((T, B, COUT), dtype=np.float32)
    for c in range(NCORES):
        u = res.results[c]["u_out"].reshape(T, 128, GC, BSH)
        s = (u == 0.0).astype(np.float32)     # spike <=> post-reset state is 0
        # out[t, 32c+bl, g*128+p] = s[t, p, g, bl]
        out[:, c * BSH:(c + 1) * BSH, :] = (
            s.transpose(0, 3, 2, 1).reshape(T, BSH, COUT))
    if _trace:
        kernel.last_exec_time_ns = res.exec_time_ns
        kernel.last_results = res
    return out


# revision 13
# speedup vs baseline: 1.0875x; 1.0875x over previous
"""PLIF (parametric LIF) spiking layer on 8 Trainium2 NeuronCores.

Computation: y = x @ W.T + b over [T=64, B=256, Cin=1024] -> Cout=1024, then a
per-timestep PLIF recurrence v = v + (y_t - v)*sigmoid(w); spike = (v >= 1);
hard reset v *= (1-spike). Output = spikes [T, B, Cout] fp32.

Strategy:
- Data-parallel over batch: core c handles b in [32c, 32c+32).
- Scaled recurrence: u_t = v_t * a^-t (a = 1-sigmoid(w)). Then u_t = u'_{t-1} + z_t
  with z_t = d*a^-t*y_t, spike iff u_t >= a^-t, reset u'_t = 0. The d*a^-t factor
  is folded into x columns on the host (exact powers of 2 when w=0), so the
  device step is ONE fused custom-DVE op:
      u' = select(u + z >= th_t, 0, u + z)
- Spikes are derived on the host as (u' == 0) - exact except measure-zero
  coincidences, which also leave the state unchanged.
- Matmul layout: out[chan, n=t*32+b] = W^T.T @ x^T. W^T resident in SBUF
  (per-k-chunk tiles so first matmuls start after ~1MB of DMA); x^T
  (host-pretransposed+scaled) streamed per (k-chunk, 512-column group);
  fp32r PE mode; PSUM evicted to a z-buffer by the scalar engine.
- mm_passes=2: split-precision GEMM - x' = xh + xl (xh = x' rounded to 12
  mantissa bits, exactly representable in the PE's fp32r input rounding), two
  accumulating passes recover ~2.4x lower spike-flip error at 2x PE cost.
"""

import numpy as np

T, B, CIN, COUT = 64, 256, 1024, 1024
NCORES = 8
BSH = B // NCORES          # 32 batch rows per core
NROWS = T * BSH            # 2048 matmul rows per core
NGROUPS = 4                # n-tile groups of 512 rows (16 timesteps each)
NG = NROWS // NGROUPS      # 512
TPG = NG // BSH            # 16 timesteps per group
KC = CIN // 128            # 8 contraction chunks
GC = COUT // 128           # 8 output-channel chunks
SFREE = GC * BSH           # 256 = state free size

_CACHE = {}


def _make_lif_op():
    import concourse.dve_ops as dve_ops
    from concourse.dve_ops import DveOp, OPS
    from concourse.dve_spec import Spec, Src0, Src1, Zero, C0, lower, select, _has_src1
    from concourse.dve_uop import DveOpSpec

    name = "LIF_STEP_ANT"
    for op in OPS:
        if op.name == name:
            return op
    def _ref(in0, in1, s0, s1, imm2):
        a = in0.reshape(in0.shape[0], -1)
        b = in1.reshape(in1.shape[0], -1)
        s = a + b
        return np.where(s >= s0, 0.0, s).astype(np.float32)

    w_ = Src0 + Src1
    spec = Spec(body=select(w_ >= C0, Zero, w_), reference=_ref)
    row = dve_ops._CUSTOM_DVE_ROW_BASE + len(OPS)
    assert row < 0x20
    shas = {}
    for ver in ("v3", "v4"):
        tmp = DveOpSpec(name=name, opcode=row, uops=lower(spec, ver=ver),
                        rd1_en=_has_src1(spec))
        shas[ver] = tmp.sha(ver)
    op = DveOp(name, spec, subdim=False, uops_sha=shas)
    OPS.append(op)
    dve_ops._SUB_OPCODE_FOR_NAME[name] = row
    dve_ops.CUSTOM_DVE_SPECS[name] = spec
    return op


def _build(thresholds, mm_dtype_name="float32r", mm_passes=1,
           x_bufs=2, z_bufs=2, u_bufs=2, psum_bufs=4, grouped_udma=True):
    import concourse.bacc as bacc
    import concourse.mybir as mybir
    import concourse.tile as tile
    from contextlib import ExitStack

    LIF = _make_lif_op()
    mm_dt = getattr(mybir.dt, mm_dtype_name)
    f32 = mybir.dt.float32

    nc = bacc.Bacc("TRN2", target_bir_lowering=False, debug=False)
    # xT holds mm_passes stacked copies (hi, then lo) along the CIN axis.
    xT_d = nc.declare_dram_parameter("xT", [mm_passes * CIN, NROWS], f32,
                                     isOutput=False)
    WT_d = nc.declare_dram_parameter("WT", [CIN, COUT], f32, isOutput=False)
    u_d = nc.declare_dram_parameter("u_out", [128, T, SFREE], f32, isOutput=True)

    xT_v = xT_d.ap().rearrange("(s c p) n -> p s c n", p=128, c=KC)
    WT_v = WT_d.ap().rearrange("(c p) o -> p c o", p=128)

    with tile.TileContext(nc) as tc:
        with ExitStack() as ctx:
            wp = ctx.enter_context(tc.tile_pool(name="wp", bufs=1))
            xp = ctx.enter_context(tc.tile_pool(name="xp", bufs=x_bufs))
            zp = ctx.enter_context(tc.tile_pool(name="zp", bufs=z_bufs))
            up = ctx.enter_context(tc.tile_pool(name="up", bufs=u_bufs))
            ip = ctx.enter_context(tc.tile_pool(name="ip", bufs=1))
            pp = ctx.enter_context(tc.tile_pool(name="pp", bufs=psum_bufs,
                                                space="PSUM"))

            u_prev = ip.tile([128, GC, BSH], f32, tag="u0")
            nc.vector.memset(u_prev[:], 0.0)

            # Per-k-chunk resident W^T tiles; interleave with group-0 x DMAs
            # so the first accumulation chain starts after ~2 chunks.
            wt = []
            xt0 = []
            for kc in range(KC):
                for s in range(mm_passes):
                    xt_ = xp.tile([128, NG], mm_dt, tag=f"xt{kc}_{s}")
                    nc.sync.dma_start(
                        xt_[:], xT_v[:, s, kc, 0:NG].bitcast(mm_dt))
                    xt0.append(xt_)
                wt_ = wp.tile([128, COUT], mm_dt, tag=f"wt{kc}")
                nc.sync.dma_start(wt_[:], WT_v[:, kc, :].bitcast(mm_dt))
                wt.append(wt_)

            for ng in range(NGROUPS):
                if ng == 0:
                    xt = xt0
                else:
                    xt = []
                    for kc in range(KC):
                        for s in range(mm_passes):
                            xt_ = xp.tile([128, NG], mm_dt, tag=f"xt{kc}_{s}")
                            nc.sync.dma_start(
                                xt_[:],
                                xT_v[:, s, kc, ng * NG:(ng + 1) * NG].bitcast(mm_dt))
                            xt.append(xt_)

                zbuf = zp.tile([128, GC, NG], f32, tag="zbuf")
                for g in range(GC):
                    psum = pp.tile([128, NG], f32, tag="ps")
                    nmm = KC * mm_passes
                    for i in range(nmm):
                        nc.tensor.matmul(
                            psum[:],
                            wt[i // mm_passes][:, g * 128:(g + 1) * 128],
                            xt[i][:],
                            start=(i == 0), stop=(i == nmm - 1))
                    nc.scalar.copy(zbuf[:, g, :], psum[:])

                ubuf = up.tile([128, TPG, GC, BSH], f32, tag="ubuf")
                for ti in range(TPG):
                    t = ng * TPG + ti
                    z_ap = zbuf[:, :, ti * BSH:(ti + 1) * BSH]
                    nc.vector._custom_dve(
                        LIF, out=ubuf[:, ti, :, :], in0=u_prev[:], in1=z_ap,
                        s0=float(thresholds[t]))
                    u_prev = ubuf[:, ti, :, :]
                    if not grouped_udma:
                        nc.sync.dma_start(
                            u_d.ap()[:, t, :].rearrange("p (g n) -> p g n", g=GC),
                            ubuf[:, ti, :, :])
                if grouped_udma:
                    nc.sync.dma_start(
                        u_d.ap()[:, ng * TPG:(ng + 1) * TPG, :],
                        ubuf[:].rearrange("p t g n -> p t (g n)"))
    nc.compile()
    return nc


def _get_nc(key, thresholds, mm_dtype_name, mm_passes, grouped_udma=True):
    if key not in _CACHE:
        _CACHE[key] = _build(thresholds, mm_dtype_name=mm_dtype_name,
                             mm_passes=mm_passes, grouped_udma=grouped_udma)
    return _CACHE[key]


def _round12(v):
    """Round fp32 to 12 mantissa bits (round-half-up in magnitude)."""
    u = v.view(np.uint32)
    add = np.uint32(1 << 10)
    return ((u + add) & np.uint32(0xFFFFF800)).view(np.float32)


def kernel(x, W, b, w, _trace=False, _mm_dtype="float32r", _mm_passes=1,
           _grouped_udma=True):
    from concourse.bass_utils import run_bass_kernel_spmd

    x = np.ascontiguousarray(np.asarray(x, dtype=np.float32))
    W = np.ascontiguousarray(np.asarray(W, dtype=np.float32))
    b = np.asarray(b, dtype=np.float32)
    wv = float(np.asarray(w, dtype=np.float32))
    assert x.shape == (T, B, CIN) and W.shape == (COUT, CIN)
    assert not np.any(b), "nonzero bias not implemented (spec fills zeros)"
    if _mm_dtype == "float32":
        _mm_passes = 1

    d = np.float64(1.0) / (np.float64(1.0) + np.exp(np.float64(-wv)))
    a = np.float64(1.0) - d
    tt = np.arange(T, dtype=np.float64)
    scales = (d * a ** (-tt)).astype(np.float32)
    thresholds = (a ** (-tt)).astype(np.float32)
    assert np.all(np.isfinite(scales)) and np.all(np.isfinite(thresholds))

    key = (_mm_dtype, _mm_passes, wv, _grouped_udma)
    nc = _get_nc(key, thresholds, _mm_dtype, _mm_passes, _grouped_udma)

    xs = x * scales[:, None, None]            # [T, B, CIN] (exact *2^k at w=0)
    WT = np.ascontiguousarray(W.T)            # [CIN, COUT]
    in_maps = []
    for c in range(NCORES):
        xc = xs[:, c * BSH:(c + 1) * BSH, :].reshape(NROWS, CIN)
        xcT = np.ascontiguousarray(xc.T)      # [CIN, NROWS]
        if _mm_passes == 2:
            xh = _round12(xcT)
            xcT = np.concatenate([xh, xcT - xh], axis=0)  # [2*CIN, NROWS]
        in_maps.append({"xT": xcT, "WT": WT})

    res = run_bass_kernel_spmd(nc, in_maps, list(range(NCORES)), trace=_trace)

    out = np.empty((T, B, COUT), dtype=np.float32)
    for c in range(NCORES):
        u = res.results[c]["u_out"].reshape(128, T, GC, BSH)
        s = (u == 0.0).astype(np.float32)     # spike <=> post-reset state is 0
        # out[t, 32c+n, g*128+p] = s[p, t, g, n]
        out[:, c * BSH:(c + 1) * BSH, :] = (
            s.transpose(1, 3, 2, 0).reshape(T, BSH, COUT))
    if _trace:
        kernel.last_exec_time_ns = res.exec_time_ns
        kernel.last_results = res
    return out
